# revision 11
# baseline (speedup 1.0000x reference)
"""Trainium2 Bass kernel for nn_MultiHeadAttention_28028956574019.

Sparse windowed multi-head attention, G=4 window groups, learned per-row
window offset. Data-parallel over batch: 8 NeuronCores, one batch element
per core.

Per-core device program (L=2048, H=1024, d=256 per group):
  offset path (folded): host precomputes woffl = off_w.T @ lin2_w.T [H,1];
      device: relu(x) (bf16, from resident qtb) -> tiny matmul -> sigmoid;
      mask row W = BIG*(q_idx + dx) broadcast to [128, 2048] via gpsimd.
  phase 2: Q/K projection (bf16); PSUM copied to bf16 QT/KT (ACT, +bias).
  phase 3: V projection (bf16): VT [l, 4*(256+1)] with ones columns.
  phase 4: per group, per 512-wide q-strip, k-blocks descending:
      S_T[k,q] = K_T.T@Q_T (2 bf16 matmuls, PSUM f32). The host computes
      dx exactly (it only depends on query+weights) and derives a block
      plan: fully-masked blocks are skipped outright; fully-inside blocks
      skip the mask and exp straight from PSUM; boundary blocks use
      z = min(S, W - BIG*(k - ws)) (DVE) with a diagonal lower-bound min,
      packed into ~1024-col z2 tiles so one ACT exp covers two blocks.
      out += p.T@V (ones col gives denominator); epilogue = reciprocal
      (DVE) + per-partition-scaled copy (ACT) + DMA.

The compiled program is cached keyed by the block plan; for a fixed input
distribution it compiles once.
"""

import sys

if "/opt/trn_rl_repo" not in sys.path:
    sys.path.insert(0, "/opt/trn_rl_repo")

import numpy as np
import ml_dtypes

import concourse.bass as bass  # noqa: F401  (bass must import before bacc)
from concourse import bacc
import concourse.mybir as mybir
from concourse.tile import TileContext
from concourse.bass_utils import run_bass_kernel_spmd

dt = mybir.dt
AF = mybir.ActivationFunctionType
Alu = mybir.AluOpType

B, L, H = 8, 2048, 1024
G, D = 4, 256          # groups, per-group head dim
D1 = 256               # learned-offset hidden dim
WS = [4, 16, 64, 256]
BIG = 1.0e7
SCALE2 = 2.0 / float(np.sqrt(L))   # masked_fill+add doubles unmasked scores
MARGIN = 16            # safety margin (keys) for host-side block decisions
NCORES = 8

_CACHE = {}


def build_nc(plan):
    """plan: dict (g, s) -> (amaxs, nmax): amaxs[j] is the highest live
    key-block for q-block 4s+j; blocks in [4s+4..nmax] need no mask."""
    nc = bacc.Bacc("TRN2", target_bir_lowering=False, debug=False)

    # ---- I/O ----  (host pre-permutes to partition-major 3D layouts so each
    # logical load is ONE dma descriptor instead of eight)
    qtbf = nc.declare_dram_parameter("qtbf", [128, 4, 8, 512], dt.bfloat16,
                                     isOutput=False)
    wqk = nc.declare_dram_parameter("wqk", [128, 16, 8, 128], dt.bfloat16,
                                    isOutput=False)
    wv = nc.declare_dram_parameter("wv", [128, 8, H], dt.bfloat16, isOutput=False)
    # consts merged into blobs to cut dma-issue serialization
    cf32 = nc.declare_dram_parameter("cf32", [128, 80], dt.float32, isOutput=False)
    cbf = nc.declare_dram_parameter("cbf", [128, 128 + H], dt.bfloat16,
                                    isOutput=False)
    wrow = nc.declare_dram_parameter("wrow", [1, L], dt.float32, isOutput=False)
    out = nc.declare_dram_parameter("out", [L, H], dt.float32, isOutput=True)

    with TileContext(nc) as tc:
        with tc.tile_pool(name="persist", bufs=1) as pp:
            # query strips first (phase 2 blocks on them); strip-major SBUF
            # layout keeps each strip dma 2D-contiguous (hw DGE on any queue).
            # strips 2,3 are issued on the sync queue inside hb==0 below
            qtball = pp.tile([128, 4, 8, 512], dt.bfloat16, name="qtball")
            for s in range(2):
                nc.gpsimd.dma_start(out=qtball[:, s, :, :], in_=qtbf[:, s, :, :])
            # qtbst[s][hin] -> [128, 512] view of query strip s, h-block hin
            qtbst = [[qtball[:, s, i, :] for i in range(8)] for s in range(4)]

            # ---- consts (three blob loads, scalar queue) ----
            cf32_t = pp.tile([128, 80], dt.float32, name="cf32_t")
            nc.scalar.dma_start(out=cf32_t[:], in_=cf32[:])
            bqk_t = cf32_t[:, 0:16]
            kvec_t = cf32_t[:, 16:80]
            wrow_t = pp.tile([1, L], dt.float32, name="wrow_t")
            nc.scalar.dma_start(out=wrow_t[:], in_=wrow[:])
            cbf_t = pp.tile([128, 128 + H], dt.bfloat16, name="cbf_t")
            nc.scalar.dma_start(out=cbf_t[:], in_=cbf[:])
            dt_t = cbf_t[:, 0:128]
            bvb = cbf_t[:, 128:128 + H]
            # strips 2,3 on the scalar queue, parallel with 0,1 on gpsimd
            for s in range(2, 4):
                nc.scalar.dma_start(out=qtball[:, s, :, :], in_=qtbf[:, s, :, :])

            # bf16 V-projection weights (needed only in phase 3), one dma
            wvall = pp.tile([128, 8, H], dt.bfloat16, name="wvall")
            nc.gpsimd.dma_start(out=wvall[:], in_=wv[:])
            wv_t = [wvall[:, i, :] for i in range(8)]

            # persistent fp8 Q_T / K_T ([128, 2, L]: both d-halves, DoubleRow)
            QT = [pp.tile([128, 2, L], dt.float8e4, name=f"QT{g}") for g in range(G)]
            KT = [pp.tile([128, 2, L], dt.float8e4, name=f"KT{g}") for g in range(G)]

            # V (natural layout) + ones column per group
            VT = []
            for lb in range(16):
                t = pp.tile([128, 4 * (D + 1)], dt.bfloat16, name=f"VT{lb}",
                            tag=f"VT{lb}")
                nc.vector.memset(t[:, D::D + 1], 1.0)
                VT.append(t)

            wbig = pp.tile([128, L], dt.float32, name="wbig")

            # HAM warm-up: dummy matmuls on a zeroed scratch tile run during
            # the input dma ramp (PE otherwise idle), so the real matmul
            # stream starts at the full 2.4 GHz clock instead of 1.2
            scr = pp.tile([128, 512], dt.bfloat16, name="scr")
            nc.vector.memset(scr[:], 0.0)
            with tc.tile_pool(name="psw", bufs=1, space="PSUM") as psw:
                wps = psw.tile([128, 512], dt.float32, name="wps")
                for _ in range(12):
                    nc.tensor.matmul(wps[:], scr[:, :128], scr[:],
                                     start=True, stop=True)

            # ============ phase 2: Q/K projection (+ offset path) ============
            # strips 2,3 arrive late; stagger their (hb, s) units a few
            # iterations behind so the PE never waits on the input dma ramp
            with tc.tile_pool(name="p2", bufs=1) as p2, \
                 tc.tile_pool(name="ps2", bufs=4, space="PSUM") as ps2:
                wts = {}

                def p2unit(hb, s):
                    wt = wts[hb]
                    g, h = (hb % 8) // 2, hb % 2
                    dest = QT[g] if hb < 8 else KT[g]
                    pps = ps2.tile([128, 512], dt.float32, tag="qkps")
                    for hin in range(8):
                        nc.tensor.matmul(pps[:], wt[:, hin, :], qtbst[s][hin],
                                         start=(hin == 0), stop=(hin == 7))
                    nc.scalar.activation(dest[:, h, s * 512:(s + 1) * 512],
                                         pps[:], AF.Identity,
                                         bias=bqk_t[:, hb:hb + 1], scale=1.0)

                for i in range(19):
                    if i <= 15:
                        wtall = p2.tile([128, 8, 128], dt.bfloat16, tag="wqk",
                                        bufs=5)
                        nc.sync.dma_start(out=wtall[:], in_=wqk[:, i, :, :])
                        wts[i] = wtall
                        p2unit(i, 0)
                        p2unit(i, 1)
                    if 2 <= i - 2 + 2 <= 17 and 0 <= i - 2 <= 15:
                        p2unit(i - 2, 2)
                    if 0 <= i - 3 <= 15:
                        p2unit(i - 3, 3)
                        wts.pop(i - 3)

                nc.gpsimd.partition_broadcast(wbig[:], wrow_t[:], channels=128)

            # ================= phase 3: V projection =================
            with tc.tile_pool(name="ps3", bufs=3, space="PSUM") as ps3:
                for lb in range(16):
                    for h in range(2):
                        vps = ps3.tile([128, 512], dt.float32, tag="vps")
                        qs, qc = lb // 4, (lb % 4) * 128
                        for hin in range(8):
                            nc.tensor.matmul(vps[:],
                                             qtbst[qs][hin][:, qc:qc + 128],
                                             wv_t[hin][:, h * 512:(h + 1) * 512],
                                             start=(hin == 0), stop=(hin == 7))
                        for gg in range(2):
                            g2 = h * 2 + gg
                            nc.vector.tensor_tensor(
                                out=VT[lb][:, g2 * (D + 1):g2 * (D + 1) + D],
                                in0=vps[:, gg * D:(gg + 1) * D],
                                in1=bvb[:, g2 * D:(g2 + 1) * D], op=Alu.add)

            # ================= phase 4: attention =================
            with tc.tile_pool(name="p4", bufs=1) as p4, \
                 tc.tile_pool(name="pss", bufs=3, space="PSUM") as pss, \
                 tc.tile_pool(name="pso", bufs=5, space="PSUM") as pso:
                ZCAP = 2048
                for g in range(G):
                    for s in (3, 2, 1, 0):
                        q0 = s * 512
                        amaxs, nmax, diag_safe = plan[(g, s)]
                        amax = max(amaxs)
                        outps = [pso.tile([128, 512], dt.float32, tag="outps",
                                          name="outps") for _ in range(4)]

                        def epilogue(j):
                            c = s * 4 + j
                            rden = p4.tile([128, 1], dt.float32, tag="rden",
                                           bufs=4, name="rden")
                            nc.vector.reciprocal(out=rden[:],
                                                 in_=outps[j][:, D:D + 1])
                            outn = p4.tile([128, D], dt.float32, tag="outn",
                                           bufs=4, name="outn")
                            if j % 2 == 0:
                                nc.scalar.mul(outn[:], outps[j][:, :D], rden[:])
                            else:
                                nc.vector.tensor_scalar(
                                    out=outn[:], in0=outps[j][:, :D],
                                    scalar1=rden[:], scalar2=None, op0=Alu.mult)
                            nc.sync.dma_start(
                                out=out[c * 128:(c + 1) * 128, g * D:(g + 1) * D],
                                in_=outn[:])

                        def consume(kb, parts):
                            for (pt, pcol, cb, jlo, jhi) in parts:
                                for j in range(jlo, jhi + 1):
                                    if 4 * s + j <= kb <= amaxs[j]:
                                        nc.tensor.matmul(
                                            outps[j][:, :D + 1],
                                            pt[:, pcol + j * 128 - cb:
                                               pcol + (j + 1) * 128 - cb],
                                            VT[kb][:, g * (D + 1):(g + 1) * (D + 1)],
                                            start=(kb == amaxs[j]),
                                            stop=(kb == 4 * s + j))
                            if kb < 4 * s + 4:
                                epilogue(kb - 4 * s)

                        def width(kb):
                            return 512 if kb >= 4 * s + 3 else (kb - 4 * s + 1) * 128

                        def loffset(kb):
                            # first live q-block for this key block
                            for j in range(4):
                                if amaxs[j] >= kb:
                                    return j * 128
                            raise AssertionError((g, s, kb, amaxs))

                        pending = []
                        z2 = None
                        zoff = 0
                        zrec = []

                        def flush_pack():
                            nonlocal z2
                            pt2 = p4.tile([128, ZCAP], dt.bfloat16, tag="pt",
                                          bufs=3, name="pt2")
                            nc.scalar.activation(pt2[:, :zoff], z2[:, :zoff],
                                                 AF.Exp, scale=SCALE2)
                            for kbx, zox, cbx, jlo, jhi, extra in zrec:
                                pending.append(
                                    (kbx, extra + [(pt2, zox, cbx, jlo, jhi)]))
                            z2 = None

                        def pack_room(need):
                            # flush if the pack can't fit `need` more columns
                            nonlocal z2, zoff, zrec
                            if z2 is not None and zoff + need > ZCAP:
                                flush_pack()
                            if z2 is None:
                                z2 = p4.tile([128, ZCAP], dt.bfloat16, tag="z",
                                             bufs=2, name="z2")
                                zoff, zrec = 0, []

                        for kb in range(amax, 4 * s - 1, -1):
                            w = width(kb)
                            off = loffset(kb)
                            jmin = off // 128
                            sps = pss.tile([128, 512], dt.float32, tag="sps")
                            nc.tensor.matmul(sps[:, off:w],
                                             KT[g][:, :, kb * 128:(kb + 1) * 128],
                                             QT[g][:, :, q0 + off:q0 + w],
                                             start=True, stop=True,
                                             perf_mode=mybir.MatmulPerfMode.DoubleRow)
                            if len(pending) >= 4:
                                consume(*pending.pop(0))
                            if 4 * s + 4 <= kb <= nmax:
                                # fully inside the window: no mask needed
                                if z2 is not None:
                                    flush_pack()
                                pt1 = p4.tile([128, 512], dt.bfloat16, tag="pt1",
                                              bufs=4, name="pt1")
                                nc.scalar.activation(pt1[:, off:w], sps[:, off:w],
                                                     AF.Exp, scale=SCALE2)
                                pending.append((kb, [(pt1, 0, 0, jmin, 3)]))
                                continue
                            if kb > nmax:
                                # boundary: windowed mask on all live columns
                                lw = w - off
                                pack_room(lw)
                                nc.vector.scalar_tensor_tensor(
                                    z2[:, zoff:zoff + lw], wbig[:, q0 + off:q0 + w],
                                    kvec_t[:, g * 16 + kb:g * 16 + kb + 1],
                                    sps[:, off:w], op0=Alu.subtract, op1=Alu.min)
                                zrec.append((kb, zoff, off, jmin, 3, []))
                                zoff += lw
                                if kb == 4 * s:
                                    flush_pack()
                                continue
                            # near-diagonal (kb <= 4s+3): upper window can't bind
                            # (host-checked diag_safe); only the triangular 128
                            # needs masking.
                            jdiag = kb - 4 * s
                            if diag_safe:
                                extra = []
                                if w - off > 128:
                                    pt1 = p4.tile([128, 512], dt.bfloat16,
                                                  tag="pt1", bufs=4, name="pt1")
                                    nc.scalar.activation(pt1[:, off:w - 128],
                                                         sps[:, off:w - 128],
                                                         AF.Exp, scale=SCALE2)
                                    extra.append((pt1, 0, 0, jmin, jdiag - 1))
                                pack_room(128)
                                nc.vector.tensor_tensor(
                                    out=z2[:, zoff:zoff + 128],
                                    in0=sps[:, w - 128:w], in1=dt_t[:], op=Alu.min)
                                zrec.append((kb, zoff, w - 128, jdiag, jdiag, extra))
                                zoff += 128
                            else:
                                lw = w - off
                                pack_room(lw)
                                nc.vector.scalar_tensor_tensor(
                                    z2[:, zoff:zoff + lw], wbig[:, q0 + off:q0 + w],
                                    kvec_t[:, g * 16 + kb:g * 16 + kb + 1],
                                    sps[:, off:w], op0=Alu.subtract, op1=Alu.min)
                                nc.vector.tensor_tensor(
                                    out=z2[:, zoff + lw - 128:zoff + lw],
                                    in0=z2[:, zoff + lw - 128:zoff + lw],
                                    in1=dt_t[:], op=Alu.min)
                                zrec.append((kb, zoff, off, jmin, jdiag, []))
                                zoff += lw
                            if kb == 4 * s:
                                flush_pack()
                        if z2 is not None:
                            flush_pack()
                        for it in pending:
                            consume(*it)

    nc.finalize()
    return nc


def _make_plan(query, woffl_np, lin2_b):
    """Host-exact window offsets -> per-(g,s) block plan (batch-uniform)."""
    z = np.maximum(query.astype(np.float64), 0.0).reshape(-1, H) @ woffl_np
    dx = (1.0 / (1.0 + np.exp(-(z + float(lin2_b[0]))))).reshape(B, L) * L
    plan = {}
    q_idx = np.arange(L, dtype=np.float64)
    for g, ws in enumerate(WS):
        lim = q_idx[None, :] + dx + ws          # [B, L] max allowed k (float)
        amax_qb = []
        for qb in range(16):
            sl = lim[:, qb * 128:(qb + 1) * 128]
            a = qb
            for kb in range(15, qb - 1, -1):
                if not (kb * 128 > sl + MARGIN).all():
                    a = kb
                    break
            amax_qb.append(a)
        for s in range(4):
            amaxs = tuple(amax_qb[4 * s:4 * s + 4])
            sl = lim[:, s * 512:(s + 1) * 512]
            nmax = 4 * s + 3
            for kb in range(min(max(amaxs), 15), 4 * s + 3, -1):
                if (kb * 128 + 127 <= sl - MARGIN).all():
                    nmax = kb
                    break
            # interior (no-mask) blocks must be live for every q-block
            assert nmax == 4 * s + 3 or nmax <= min(amaxs), (g, s, amaxs, nmax)
            # near-diagonal blocks (kb<=4s+3, k-q<=511) can skip the upper
            # window test iff the window covers >=511+MARGIN keys for every row
            diag_safe = bool(
                (dx[:, s * 512:(s + 1) * 512] + ws > 511 + MARGIN).all())
            plan[(g, s)] = (amaxs, nmax, diag_safe)
    return plan, dx


def _prep_shared(qkv_w, qkv_b, off_w, lin2_w, lin2_b):
    f32 = np.float32
    bf = ml_dtypes.bfloat16
    qkv_wT = np.ascontiguousarray(qkv_w.T, dtype=f32)          # [H, 3H]
    woffl = (off_w.T.astype(np.float64) @ lin2_w.T.astype(np.float64))  # [H, 1]
    # [H, 2H] -> [p, hb, hin, c]; [H, H] -> [p, hin, c]  (partition-major)
    wqk_np = (qkv_wT[:, :2 * H].reshape(8, 128, 16, 128)
              .transpose(1, 2, 0, 3))
    wv_np = qkv_wT[:, 2 * H:].reshape(8, 128, H).transpose(1, 0, 2)
    p = np.arange(128, dtype=np.float64)[:, None]
    cols = []
    for g in range(G):
        for kb in range(16):
            cols.append(BIG * (kb * 128 + p - WS[g]))
    kvec = np.concatenate(cols, axis=1).astype(f32)
    bqk = np.ascontiguousarray(qkv_b[:2 * H].reshape(16, 128).T, dtype=f32)
    pi = np.arange(128)[:, None]
    fi = np.arange(128)[None, :]
    dtile = np.where(pi >= fi, 1e6, -1e6).astype(f32)
    bv = np.broadcast_to(qkv_b[2 * H:][None], (128, H))
    woffl_col = woffl.reshape(8, 128).T
    iotab = BIG * np.arange(L, dtype=np.float64)
    shared = {
        "wqk": np.ascontiguousarray(wqk_np).astype(bf),
        "wv": np.ascontiguousarray(wv_np).astype(bf),
        "cf32": np.concatenate([bqk, kvec], axis=1).astype(f32),
        "cbf": np.concatenate([dtile, bv], axis=1).astype(bf),
    }
    return shared, woffl


def kernel(query, key_in, value, qkv_w, qkv_b, off_w, lin2_w, lin2_b,
           _trace=False, _tmpdir=None):
    query = np.asarray(query, dtype=np.float32)
    shared, woffl_np = _prep_shared(np.asarray(qkv_w, np.float32),
                                    np.asarray(qkv_b, np.float32),
                                    np.asarray(off_w, np.float32),
                                    np.asarray(lin2_w, np.float32),
                                    np.asarray(lin2_b, np.float32))
    plan, dx = _make_plan(query, woffl_np, np.asarray(lin2_b, np.float64).ravel())
    in_maps = []
    for b in range(NCORES):
        m = dict(shared)
        # [p, strip, hin, col] so each 512-col strip is one contiguous dma run
        qT = (query[b].T.reshape(8, 128, 4, 512).transpose(1, 2, 0, 3))
        m["qtbf"] = np.ascontiguousarray(qT).astype(ml_dtypes.bfloat16)
        m["wrow"] = (BIG * (np.arange(L, dtype=np.float64) + dx[b])
                     ).astype(np.float32)[None]
        in_maps.append(m)

    key = tuple(sorted(plan.items()))
    if key not in _CACHE:
        _CACHE[key] = build_nc(plan)
    kw = {}
    if _trace:
        kw = dict(trace=True, tmpdir=_tmpdir)
    res = run_bass_kernel_spmd(_CACHE[key], in_maps,
                               core_ids=list(range(NCORES)), **kw)
    out = np.stack([np.asarray(res.results[b]["out"]) for b in range(NCORES)],
                   axis=0)
    if _trace:
        return out, res
    return out


if __name__ == "__main__":
    rng = np.random.default_rng(0)
    ins = {
        "query": rng.standard_normal((B, L, H)).astype(np.float32),
        "key_in": rng.standard_normal((B, L, H)).astype(np.float32),
        "value": rng.standard_normal((B, L, H)).astype(np.float32),
        "qkv_w": (rng.standard_normal((3 * H, H)) * 0.02).astype(np.float32),
        "qkv_b": np.zeros(3 * H, np.float32),
        "off_w": (rng.standard_normal((D1, H)) * 0.02).astype(np.float32),
        "lin2_w": (rng.standard_normal((1, D1)) * 0.02).astype(np.float32),
        "lin2_b": np.zeros(1, np.float32),
    }
    o = kernel(**ins)
    print("out", o.shape, o.dtype, np.abs(o).mean())



# revision 13
# speedup vs baseline: 1.0496x; 1.0496x over previous
"""Trainium2 Bass kernel for nn_MultiHeadAttention_28028956574019.

Sparse windowed multi-head attention, G=4 window groups, learned per-row
window offset. Data-parallel over batch: 8 NeuronCores, one batch element
per core.

Per-core device program (L=2048, H=1024, d=256 per group):
  offset path (folded): host precomputes woffl = off_w.T @ lin2_w.T [H,1];
      device: relu(x) (bf16, from resident qtb) -> tiny matmul -> sigmoid;
      mask row W = BIG*(q_idx + dx) broadcast to [128, 2048] via gpsimd.
  phase 2: Q/K projection (bf16); PSUM copied to bf16 QT/KT (ACT, +bias).
  phase 3: V projection (bf16): VT [l, 4*(256+1)] with ones columns.
  phase 4: per group, per 512-wide q-strip, k-blocks descending:
      S_T[k,q] = K_T.T@Q_T (2 bf16 matmuls, PSUM f32). The host computes
      dx exactly (it only depends on query+weights) and derives a block
      plan: fully-masked blocks are skipped outright; fully-inside blocks
      skip the mask and exp straight from PSUM; boundary blocks use
      z = min(S, W - BIG*(k - ws)) (DVE) with a diagonal lower-bound min,
      packed into ~1024-col z2 tiles so one ACT exp covers two blocks.
      out += p.T@V (ones col gives denominator); epilogue = reciprocal
      (DVE) + per-partition-scaled copy (ACT) + DMA.

The compiled program is cached keyed by the block plan; for a fixed input
distribution it compiles once.
"""

import sys

if "/opt/trn_rl_repo" not in sys.path:
    sys.path.insert(0, "/opt/trn_rl_repo")

import numpy as np
import ml_dtypes

import concourse.bass as bass  # noqa: F401  (bass must import before bacc)
from concourse import bacc
import concourse.mybir as mybir
from concourse.tile import TileContext
from concourse.bass_utils import run_bass_kernel_spmd

dt = mybir.dt
AF = mybir.ActivationFunctionType
Alu = mybir.AluOpType

B, L, H = 8, 2048, 1024
G, D = 4, 256          # groups, per-group head dim
D1 = 256               # learned-offset hidden dim
WS = [4, 16, 64, 256]
BIG = 1.0e7
SCALE2 = 2.0 / float(np.sqrt(L))   # masked_fill+add doubles unmasked scores
MARGIN = 16            # safety margin (keys) for host-side block decisions
NCORES = 8

_CACHE = {}


def build_nc(plan):
    """plan: dict (g, s) -> (amaxs, nmax): amaxs[j] is the highest live
    key-block for q-block 4s+j; blocks in [4s+4..nmax] need no mask."""
    nc = bacc.Bacc("TRN2", target_bir_lowering=False, debug=False)

    # ---- I/O ----  (host pre-permutes to partition-major 3D layouts so each
    # logical load is ONE dma descriptor instead of eight)
    qtbf = nc.declare_dram_parameter("qtbf", [128, 4, 8, 512], dt.bfloat16,
                                     isOutput=False)
    wqk = nc.declare_dram_parameter("wqk", [128, 16, 8, 128], dt.bfloat16,
                                    isOutput=False)
    wv = nc.declare_dram_parameter("wv", [128, 8, H], dt.bfloat16, isOutput=False)
    # consts merged into blobs to cut dma-issue serialization
    cf32 = nc.declare_dram_parameter("cf32", [128, 80], dt.float32, isOutput=False)
    cbf = nc.declare_dram_parameter("cbf", [128, 128 + H], dt.bfloat16,
                                    isOutput=False)
    wrow = nc.declare_dram_parameter("wrow", [1, L], dt.float32, isOutput=False)
    out = nc.declare_dram_parameter("out", [L, H], dt.float32, isOutput=True)

    with TileContext(nc) as tc:
        with tc.tile_pool(name="persist", bufs=1) as pp:
            # query strips first (phase 2 blocks on them); strip-major SBUF
            # layout keeps each strip dma 2D-contiguous (hw DGE on any queue).
            # strips 2,3 are issued on the sync queue inside hb==0 below
            qtball = pp.tile([128, 4, 8, 512], dt.bfloat16, name="qtball")
            for s in range(2):
                nc.gpsimd.dma_start(out=qtball[:, s, :, :], in_=qtbf[:, s, :, :])
            # qtbst[s][hin] -> [128, 512] view of query strip s, h-block hin
            qtbst = [[qtball[:, s, i, :] for i in range(8)] for s in range(4)]

            # ---- consts (three blob loads, scalar queue) ----
            cf32_t = pp.tile([128, 80], dt.float32, name="cf32_t")
            nc.scalar.dma_start(out=cf32_t[:], in_=cf32[:])
            bqk_t = cf32_t[:, 0:16]
            kvec_t = cf32_t[:, 16:80]
            wrow_t = pp.tile([1, L], dt.float32, name="wrow_t")
            nc.scalar.dma_start(out=wrow_t[:], in_=wrow[:])
            cbf_t = pp.tile([128, 128 + H], dt.bfloat16, name="cbf_t")
            nc.scalar.dma_start(out=cbf_t[:], in_=cbf[:])
            dt_t = cbf_t[:, 0:128]
            bvb = cbf_t[:, 128:128 + H]
            # strips 2,3 on the scalar queue, parallel with 0,1 on gpsimd
            for s in range(2, 4):
                nc.scalar.dma_start(out=qtball[:, s, :, :], in_=qtbf[:, s, :, :])

            # bf16 V-projection weights (needed only in phase 3), one dma
            wvall = pp.tile([128, 8, H], dt.bfloat16, name="wvall")
            nc.gpsimd.dma_start(out=wvall[:], in_=wv[:])
            wv_t = [wvall[:, i, :] for i in range(8)]

            # persistent fp8 Q_T / K_T ([128, 2, L]: both d-halves, DoubleRow)
            QT = [pp.tile([128, 2, L], dt.float8e4, name=f"QT{g}") for g in range(G)]
            KT = [pp.tile([128, 2, L], dt.float8e4, name=f"KT{g}") for g in range(G)]

            # V (natural layout) + ones column per group
            VT = []
            for lb in range(16):
                t = pp.tile([128, 4 * (D + 1)], dt.bfloat16, name=f"VT{lb}",
                            tag=f"VT{lb}")
                nc.vector.memset(t[:, D::D + 1], 1.0)
                VT.append(t)

            wbig = pp.tile([128, L], dt.float32, name="wbig")

            # HAM warm-up: dummy matmuls on a zeroed scratch tile run during
            # the input dma ramp (PE otherwise idle), so the real matmul
            # stream starts at the full 2.4 GHz clock instead of 1.2
            scr = pp.tile([128, 512], dt.bfloat16, name="scr")
            nc.vector.memset(scr[:], 0.0)
            with tc.tile_pool(name="psw", bufs=1, space="PSUM") as psw:
                wps = psw.tile([128, 512], dt.float32, name="wps")
                for _ in range(12):
                    nc.tensor.matmul(wps[:], scr[:, :128], scr[:],
                                     start=True, stop=True)

            # ===== interleaved emission: QK proj g0, then attention(g)
            # with V-proj / QK-proj(g+1) units woven between S blocks so
            # the exp/mask (ACT/DVE) load spreads over the whole timeline
            with tc.tile_pool(name="p2", bufs=1) as p2, \
                 tc.tile_pool(name="p4", bufs=1) as p4, \
                 tc.tile_pool(name="ps2", bufs=2, space="PSUM") as ps2, \
                 tc.tile_pool(name="pss", bufs=2, space="PSUM") as pss, \
                 tc.tile_pool(name="pso", bufs=4, space="PSUM") as pso:
                wts = {}

                def wqk_load(hb):
                    wtall = p2.tile([128, 8, 128], dt.bfloat16, tag="wqk",
                                    bufs=5)
                    nc.sync.dma_start(out=wtall[:], in_=wqk[:, hb, :, :])
                    wts[hb] = wtall

                def p2unit(hb, s):
                    wt = wts[hb]
                    g, h = (hb % 8) // 2, hb % 2
                    dest = QT[g] if hb < 8 else KT[g]
                    pps = ps2.tile([128, 512], dt.float32, tag="qkps")
                    for hin in range(8):
                        nc.tensor.matmul(pps[:], wt[:, hin, :], qtbst[s][hin],
                                         start=(hin == 0), stop=(hin == 7))
                    nc.scalar.activation(dest[:, h, s * 512:(s + 1) * 512],
                                         pps[:], AF.Identity,
                                         bias=bqk_t[:, hb:hb + 1], scale=1.0)
                    if s == 3:
                        wts.pop(hb)

                def vunit(lb, h):
                    vps = ps2.tile([128, 512], dt.float32, tag="qkps")
                    qs, qc = lb // 4, (lb % 4) * 128
                    for hin in range(8):
                        nc.tensor.matmul(vps[:],
                                         qtbst[qs][hin][:, qc:qc + 128],
                                         wv_t[hin][:, h * 512:(h + 1) * 512],
                                         start=(hin == 0), stop=(hin == 7))
                    for gg in range(2):
                        g2 = h * 2 + gg
                        nc.vector.tensor_tensor(
                            out=VT[lb][:, g2 * (D + 1):g2 * (D + 1) + D],
                            in0=vps[:, gg * D:(gg + 1) * D],
                            in1=bvb[:, g2 * D:(g2 + 1) * D], op=Alu.add)

                def emit(u):
                    if u[0] == "load":
                        wqk_load(u[1])
                    elif u[0] == "unit":
                        p2unit(u[1], u[2])
                    else:
                        vunit(u[1], u[2])

                ghb = [[2 * g, 2 * g + 1, 8 + 2 * g, 8 + 2 * g + 1]
                       for g in range(G)]

                # ---- QK proj for group 0 (staggered for the dma ramp) ----
                hbs = ghb[0]
                for i, hb in enumerate(hbs):
                    wqk_load(hb)
                    p2unit(hb, 0)
                    p2unit(hb, 1)
                    if i >= 2:
                        p2unit(hbs[i - 2], 2)
                    if i >= 3:
                        p2unit(hbs[i - 3], 3)
                p2unit(hbs[2], 2)
                p2unit(hbs[1], 3)
                p2unit(hbs[3], 2)
                p2unit(hbs[2], 3)
                p2unit(hbs[3], 3)

                nc.gpsimd.partition_broadcast(wbig[:], wrow_t[:], channels=128)

                # ---- filler queue: V-proj + later groups' QK proj, with
                # deadlines (unit must be emitted before deadline pair starts)
                fillq = []  # (deadline_pair_index, unit)

                def pidx(g, s):
                    return g * 4 + (3 - s)

                for lb in range(15, -1, -1):
                    for h in range(2):
                        # VT[lb] consumed from pair (0, lb//4) onward
                        fillq.append((pidx(0, lb // 4), ("v", lb, h)))
                for gg in range(1, G):
                    dl = pidx(gg, 3)
                    h2 = ghb[gg]
                    fillq.append((dl, ("load", h2[0])))
                    fillq.append((dl, ("load", h2[1])))
                    for idx, hb in enumerate(h2):
                        if idx + 2 < 4:
                            fillq.append((dl, ("load", h2[idx + 2])))
                        for s in range(4):
                            fillq.append((dl, ("unit", hb, s)))

                ZCAP = 2048
                for g in range(G):
                    for s in (3, 2, 1, 0):
                        # force-emit fillers due before this pair
                        while fillq and fillq[0][0] <= pidx(g, s):
                            emit(fillq.pop(0)[1])

                        q0 = s * 512
                        amaxs, nmax, diag_safe = plan[(g, s)]
                        amax = max(amaxs)
                        outps = [pso.tile([128, 512], dt.float32, tag="outps",
                                          name="outps") for _ in range(4)]

                        def epilogue(j):
                            c = s * 4 + j
                            rden = p4.tile([128, 1], dt.float32, tag="rden",
                                           bufs=4, name="rden")
                            nc.vector.reciprocal(out=rden[:],
                                                 in_=outps[j][:, D:D + 1])
                            outn = p4.tile([128, D], dt.float32, tag="outn",
                                           bufs=4, name="outn")
                            if j % 2 == 0:
                                nc.scalar.mul(outn[:], outps[j][:, :D], rden[:])
                            else:
                                nc.vector.tensor_scalar(
                                    out=outn[:], in0=outps[j][:, :D],
                                    scalar1=rden[:], scalar2=None, op0=Alu.mult)
                            nc.sync.dma_start(
                                out=out[c * 128:(c + 1) * 128, g * D:(g + 1) * D],
                                in_=outn[:])

                        def consume(kb, parts):
                            for (pt, pcol, cb, jlo, jhi) in parts:
                                for j in range(jlo, jhi + 1):
                                    if 4 * s + j <= kb <= amaxs[j]:
                                        nc.tensor.matmul(
                                            outps[j][:, :D + 1],
                                            pt[:, pcol + j * 128 - cb:
                                               pcol + (j + 1) * 128 - cb],
                                            VT[kb][:, g * (D + 1):(g + 1) * (D + 1)],
                                            start=(kb == amaxs[j]),
                                            stop=(kb == 4 * s + j))
                            if kb < 4 * s + 4:
                                epilogue(kb - 4 * s)

                        def width(kb):
                            return 512 if kb >= 4 * s + 3 else (kb - 4 * s + 1) * 128

                        def loffset(kb):
                            # first live q-block for this key block
                            for j in range(4):
                                if amaxs[j] >= kb:
                                    return j * 128
                            raise AssertionError((g, s, kb, amaxs))

                        pending = []
                        z2 = None
                        zoff = 0
                        zrec = []

                        def flush_pack():
                            nonlocal z2
                            pt2 = p4.tile([128, ZCAP], dt.bfloat16, tag="pt",
                                          bufs=3, name="pt2")
                            nc.scalar.activation(pt2[:, :zoff], z2[:, :zoff],
                                                 AF.Exp, scale=SCALE2)
                            for kbx, zox, cbx, jlo, jhi, extra in zrec:
                                pending.append(
                                    (kbx, extra + [(pt2, zox, cbx, jlo, jhi)]))
                            z2 = None

                        def pack_room(need):
                            # flush if the pack can't fit `need` more columns
                            nonlocal z2, zoff, zrec
                            if z2 is not None and zoff + need > ZCAP:
                                flush_pack()
                            if z2 is None:
                                z2 = p4.tile([128, ZCAP], dt.bfloat16, tag="z",
                                             bufs=2, name="z2")
                                zoff, zrec = 0, []

                        for kb in range(amax, 4 * s - 1, -1):
                            w = width(kb)
                            off = loffset(kb)
                            jmin = off // 128
                            sps = pss.tile([128, 512], dt.float32, tag="sps")
                            nc.tensor.matmul(sps[:, off:w],
                                             KT[g][:, :, kb * 128:(kb + 1) * 128],
                                             QT[g][:, :, q0 + off:q0 + w],
                                             start=True, stop=True,
                                             perf_mode=mybir.MatmulPerfMode.DoubleRow)
                            if len(pending) >= 4:
                                consume(*pending.pop(0))
                            if fillq:
                                emit(fillq.pop(0)[1])
                            if 4 * s + 4 <= kb <= nmax:
                                # fully inside the window: no mask needed
                                if z2 is not None:
                                    flush_pack()
                                pt1 = p4.tile([128, 512], dt.bfloat16, tag="pt1",
                                              bufs=4, name="pt1")
                                nc.scalar.activation(pt1[:, off:w], sps[:, off:w],
                                                     AF.Exp, scale=SCALE2)
                                pending.append((kb, [(pt1, 0, 0, jmin, 3)]))
                                continue
                            if kb > nmax:
                                # boundary: windowed mask on all live columns
                                lw = w - off
                                pack_room(lw)
                                nc.vector.scalar_tensor_tensor(
                                    z2[:, zoff:zoff + lw], wbig[:, q0 + off:q0 + w],
                                    kvec_t[:, g * 16 + kb:g * 16 + kb + 1],
                                    sps[:, off:w], op0=Alu.subtract, op1=Alu.min)
                                zrec.append((kb, zoff, off, jmin, 3, []))
                                zoff += lw
                                if kb == 4 * s:
                                    flush_pack()
                                continue
                            # near-diagonal (kb <= 4s+3): upper window can't bind
                            # (host-checked diag_safe); only the triangular 128
                            # needs masking.
                            jdiag = kb - 4 * s
                            if diag_safe:
                                extra = []
                                if w - off > 128:
                                    pt1 = p4.tile([128, 512], dt.bfloat16,
                                                  tag="pt1", bufs=4, name="pt1")
                                    nc.scalar.activation(pt1[:, off:w - 128],
                                                         sps[:, off:w - 128],
                                                         AF.Exp, scale=SCALE2)
                                    extra.append((pt1, 0, 0, jmin, jdiag - 1))
                                pack_room(128)
                                nc.vector.tensor_tensor(
                                    out=z2[:, zoff:zoff + 128],
                                    in0=sps[:, w - 128:w], in1=dt_t[:], op=Alu.min)
                                zrec.append((kb, zoff, w - 128, jdiag, jdiag, extra))
                                zoff += 128
                            else:
                                lw = w - off
                                pack_room(lw)
                                nc.vector.scalar_tensor_tensor(
                                    z2[:, zoff:zoff + lw], wbig[:, q0 + off:q0 + w],
                                    kvec_t[:, g * 16 + kb:g * 16 + kb + 1],
                                    sps[:, off:w], op0=Alu.subtract, op1=Alu.min)
                                nc.vector.tensor_tensor(
                                    out=z2[:, zoff + lw - 128:zoff + lw],
                                    in0=z2[:, zoff + lw - 128:zoff + lw],
                                    in1=dt_t[:], op=Alu.min)
                                zrec.append((kb, zoff, off, jmin, jdiag, []))
                                zoff += lw
                            if kb == 4 * s:
                                flush_pack()
                        if z2 is not None:
                            flush_pack()
                        for it in pending:
                            consume(*it)

    nc.finalize()
    return nc


def _make_plan(query, woffl_np, lin2_b):
    """Host-exact window offsets -> per-(g,s) block plan (batch-uniform)."""
    z = np.maximum(query.astype(np.float64), 0.0).reshape(-1, H) @ woffl_np
    dx = (1.0 / (1.0 + np.exp(-(z + float(lin2_b[0]))))).reshape(B, L) * L
    plan = {}
    q_idx = np.arange(L, dtype=np.float64)
    for g, ws in enumerate(WS):
        lim = q_idx[None, :] + dx + ws          # [B, L] max allowed k (float)
        amax_qb = []
        for qb in range(16):
            sl = lim[:, qb * 128:(qb + 1) * 128]
            a = qb
            for kb in range(15, qb - 1, -1):
                if not (kb * 128 > sl + MARGIN).all():
                    a = kb
                    break
            amax_qb.append(a)
        for s in range(4):
            amaxs = tuple(amax_qb[4 * s:4 * s + 4])
            sl = lim[:, s * 512:(s + 1) * 512]
            nmax = 4 * s + 3
            for kb in range(min(max(amaxs), 15), 4 * s + 3, -1):
                if (kb * 128 + 127 <= sl - MARGIN).all():
                    nmax = kb
                    break
            # interior (no-mask) blocks must be live for every q-block
            assert nmax == 4 * s + 3 or nmax <= min(amaxs), (g, s, amaxs, nmax)
            # near-diagonal blocks (kb<=4s+3, k-q<=511) can skip the upper
            # window test iff the window covers >=511+MARGIN keys for every row
            diag_safe = bool(
                (dx[:, s * 512:(s + 1) * 512] + ws > 511 + MARGIN).all())
            plan[(g, s)] = (amaxs, nmax, diag_safe)
    return plan, dx


def _prep_shared(qkv_w, qkv_b, off_w, lin2_w, lin2_b):
    f32 = np.float32
    bf = ml_dtypes.bfloat16
    qkv_wT = np.ascontiguousarray(qkv_w.T, dtype=f32)          # [H, 3H]
    woffl = (off_w.T.astype(np.float64) @ lin2_w.T.astype(np.float64))  # [H, 1]
    # [H, 2H] -> [p, hb, hin, c]; [H, H] -> [p, hin, c]  (partition-major)
    wqk_np = (qkv_wT[:, :2 * H].reshape(8, 128, 16, 128)
              .transpose(1, 2, 0, 3))
    wv_np = qkv_wT[:, 2 * H:].reshape(8, 128, H).transpose(1, 0, 2)
    p = np.arange(128, dtype=np.float64)[:, None]
    cols = []
    for g in range(G):
        for kb in range(16):
            cols.append(BIG * (kb * 128 + p - WS[g]))
    kvec = np.concatenate(cols, axis=1).astype(f32)
    bqk = np.ascontiguousarray(qkv_b[:2 * H].reshape(16, 128).T, dtype=f32)
    pi = np.arange(128)[:, None]
    fi = np.arange(128)[None, :]
    dtile = np.where(pi >= fi, 1e6, -1e6).astype(f32)
    bv = np.broadcast_to(qkv_b[2 * H:][None], (128, H))
    woffl_col = woffl.reshape(8, 128).T
    iotab = BIG * np.arange(L, dtype=np.float64)
    shared = {
        "wqk": np.ascontiguousarray(wqk_np).astype(bf),
        "wv": np.ascontiguousarray(wv_np).astype(bf),
        "cf32": np.concatenate([bqk, kvec], axis=1).astype(f32),
        "cbf": np.concatenate([dtile, bv], axis=1).astype(bf),
    }
    return shared, woffl


def kernel(query, key_in, value, qkv_w, qkv_b, off_w, lin2_w, lin2_b,
           _trace=False, _tmpdir=None):
    query = np.asarray(query, dtype=np.float32)
    shared, woffl_np = _prep_shared(np.asarray(qkv_w, np.float32),
                                    np.asarray(qkv_b, np.float32),
                                    np.asarray(off_w, np.float32),
                                    np.asarray(lin2_w, np.float32),
                                    np.asarray(lin2_b, np.float32))
    plan, dx = _make_plan(query, woffl_np, np.asarray(lin2_b, np.float64).ravel())
    in_maps = []
    for b in range(NCORES):
        m = dict(shared)
        # [p, strip, hin, col] so each 512-col strip is one contiguous dma run
        qT = (query[b].T.reshape(8, 128, 4, 512).transpose(1, 2, 0, 3))
        m["qtbf"] = np.ascontiguousarray(qT).astype(ml_dtypes.bfloat16)
        m["wrow"] = (BIG * (np.arange(L, dtype=np.float64) + dx[b])
                     ).astype(np.float32)[None]
        in_maps.append(m)

    key = tuple(sorted(plan.items()))
    if key not in _CACHE:
        _CACHE[key] = build_nc(plan)
    kw = {}
    if _trace:
        kw = dict(trace=True, tmpdir=_tmpdir)
    res = run_bass_kernel_spmd(_CACHE[key], in_maps,
                               core_ids=list(range(NCORES)), **kw)
    out = np.stack([np.asarray(res.results[b]["out"]) for b in range(NCORES)],
                   axis=0)
    if _trace:
        return out, res
    return out


if __name__ == "__main__":
    rng = np.random.default_rng(0)
    ins = {
        "query": rng.standard_normal((B, L, H)).astype(np.float32),
        "key_in": rng.standard_normal((B, L, H)).astype(np.float32),
        "value": rng.standard_normal((B, L, H)).astype(np.float32),
        "qkv_w": (rng.standard_normal((3 * H, H)) * 0.02).astype(np.float32),
        "qkv_b": np.zeros(3 * H, np.float32),
        "off_w": (rng.standard_normal((D1, H)) * 0.02).astype(np.float32),
        "lin2_w": (rng.standard_normal((1, D1)) * 0.02).astype(np.float32),
        "lin2_b": np.zeros(1, np.float32),
    }
    o = kernel(**ins)
    print("out", o.shape, o.dtype, np.abs(o).mean())



# revision 20
# speedup vs baseline: 1.0928x; 1.0411x over previous
"""Trainium2 Bass kernel for nn_MultiHeadAttention_28028956574019.

Sparse windowed multi-head attention, G=4 window groups, learned per-row
window offset. Data-parallel over batch: 8 NeuronCores, one batch element
per core.

Per-core device program (L=2048, H=1024, d=256 per group):
  offset path (folded): host precomputes woffl = off_w.T @ lin2_w.T [H,1];
      device: relu(x) (bf16, from resident qtb) -> tiny matmul -> sigmoid;
      mask row W = BIG*(q_idx + dx) broadcast to [128, 2048] via gpsimd.
  phase 2: Q/K projection (bf16); PSUM copied to bf16 QT/KT (ACT, +bias).
  phase 3: V projection (bf16): VT [l, 4*(256+1)] with ones columns.
  phase 4: per group, per 512-wide q-strip, k-blocks descending:
      S_T[k,q] = K_T.T@Q_T (2 bf16 matmuls, PSUM f32). The host computes
      dx exactly (it only depends on query+weights) and derives a block
      plan: fully-masked blocks are skipped outright; fully-inside blocks
      skip the mask and exp straight from PSUM; boundary blocks use
      z = min(S, W - BIG*(k - ws)) (DVE) with a diagonal lower-bound min,
      packed into ~1024-col z2 tiles so one ACT exp covers two blocks.
      out += p.T@V (ones col gives denominator); epilogue = reciprocal
      (DVE) + per-partition-scaled copy (ACT) + DMA.

The compiled program is cached keyed by the block plan; for a fixed input
distribution it compiles once.
"""

import sys

if "/opt/trn_rl_repo" not in sys.path:
    sys.path.insert(0, "/opt/trn_rl_repo")

import numpy as np
import ml_dtypes

import concourse.bass as bass  # noqa: F401  (bass must import before bacc)
from concourse import bacc
import concourse.mybir as mybir
from concourse.tile import TileContext
from concourse.bass_utils import run_bass_kernel_spmd

dt = mybir.dt
AF = mybir.ActivationFunctionType
Alu = mybir.AluOpType

B, L, H = 8, 2048, 1024
G, D = 4, 256          # groups, per-group head dim
D1 = 256               # learned-offset hidden dim
WS = [4, 16, 64, 256]
BIG = 1.0e7
SCALE2 = 2.0 / float(np.sqrt(L))   # masked_fill+add doubles unmasked scores
MARGIN = 16            # safety margin (keys) for host-side block decisions
NCORES = 8

_CACHE = {}


def build_nc(plan):
    """plan: dict (g, s) -> (amaxs, nmax): amaxs[j] is the highest live
    key-block for q-block 4s+j; blocks in [4s+4..nmax] need no mask."""
    nc = bacc.Bacc("TRN2", target_bir_lowering=False, debug=False)

    # ---- I/O ----  (host pre-permutes to partition-major 3D layouts so each
    # logical load is ONE dma descriptor instead of eight)
    qtbf = nc.declare_dram_parameter("qtbf", [128, 4, 8, 512], dt.bfloat16,
                                     isOutput=False)
    # fp8 copies (scaled x8) for the DoubleRow Q projection
    qtb8 = nc.declare_dram_parameter("qtb8", [128, 4, 4, 2, 512], dt.float8e4,
                                     isOutput=False)
    wq8 = nc.declare_dram_parameter("wq8", [128, 8, 4, 2, 128], dt.float8e4,
                                    isOutput=False)
    wqk = nc.declare_dram_parameter("wqk", [128, 16, 8, 128], dt.bfloat16,
                                    isOutput=False)
    wv = nc.declare_dram_parameter("wv", [128, 8, H], dt.bfloat16, isOutput=False)
    # consts merged into blobs to cut dma-issue serialization
    cf32 = nc.declare_dram_parameter("cf32", [128, 80], dt.float32, isOutput=False)
    cbf = nc.declare_dram_parameter("cbf", [128, 128 + H], dt.bfloat16,
                                    isOutput=False)
    wrow = nc.declare_dram_parameter("wrow", [1, L], dt.float32, isOutput=False)
    out = nc.declare_dram_parameter("out", [L, H], dt.float32, isOutput=True)

    with TileContext(nc) as tc:
        with tc.tile_pool(name="persist", bufs=1) as pp:
            # query strips first (phase 2 blocks on them); strip-major SBUF
            # layout keeps each strip dma 2D-contiguous (hw DGE on any queue).
            # strips 2,3 are issued on the sync queue inside hb==0 below
            # fp8 query strips first (Q proj units run first), then bf16
            # strips for the K/V projections on the scalar queue
            qtb8t = pp.tile([128, 4, 4, 2, 512], dt.float8e4, name="qtb8t")
            for s in range(4):
                nc.gpsimd.dma_start(out=qtb8t[:, s, :, :, :],
                                    in_=qtb8[:, s, :, :, :])
            qtball = pp.tile([128, 4, 8, 512], dt.bfloat16, name="qtball")
            # qtbst[s][hin] -> [128, 512] view of query strip s, h-block hin
            qtbst = [[qtball[:, s, i, :] for i in range(8)] for s in range(4)]

            # ---- consts (three blob loads, scalar queue) ----
            cf32_t = pp.tile([128, 80], dt.float32, name="cf32_t")
            nc.scalar.dma_start(out=cf32_t[:], in_=cf32[:])
            bqk_t = cf32_t[:, 0:16]
            kvec_t = cf32_t[:, 16:80]
            wrow_t = pp.tile([1, L], dt.float32, name="wrow_t")
            nc.scalar.dma_start(out=wrow_t[:], in_=wrow[:])
            for s in range(4):
                nc.scalar.dma_start(out=qtball[:, s, :, :], in_=qtbf[:, s, :, :])
            cbf_t = pp.tile([128, 128 + H], dt.bfloat16, name="cbf_t")
            nc.scalar.dma_start(out=cbf_t[:], in_=cbf[:])
            dt_t = cbf_t[:, 0:128]
            bvb = cbf_t[:, 128:128 + H]

            # bf16 V-projection weights (needed only in phase 3), one dma
            wvall = pp.tile([128, 8, H], dt.bfloat16, name="wvall")
            nc.gpsimd.dma_start(out=wvall[:], in_=wv[:])
            wv_t = [wvall[:, i, :] for i in range(8)]

            # persistent fp8 Q_T / K_T ([128, 2, L]: both d-halves, DoubleRow)
            QT = [pp.tile([128, 2, L], dt.float8e4, name=f"QT{g}") for g in range(G)]
            KT = [pp.tile([128, 2, L], dt.float8e4, name=f"KT{g}") for g in range(G)]

            # V (natural layout) + ones column per group
            VT = []
            for lb in range(16):
                t = pp.tile([128, 4 * (D + 1)], dt.bfloat16, name=f"VT{lb}",
                            tag=f"VT{lb}")
                nc.vector.memset(t[:, D::D + 1], 1.0)
                VT.append(t)

            wbig = pp.tile([128, L], dt.float32, name="wbig")

            # HAM warm-up: dummy matmuls on a zeroed scratch tile run during
            # the input dma ramp (PE otherwise idle), so the real matmul
            # stream starts at the full 2.4 GHz clock instead of 1.2
            scr = pp.tile([128, 512], dt.bfloat16, name="scr")
            nc.vector.memset(scr[:], 0.0)
            with tc.tile_pool(name="psw", bufs=1, space="PSUM") as psw:
                wps = psw.tile([128, 512], dt.float32, name="wps")
                for _ in range(12):
                    nc.tensor.matmul(wps[:], scr[:, :128], scr[:],
                                     start=True, stop=True)

            # ===== interleaved emission: QK proj g0, then attention(g)
            # with V-proj / QK-proj(g+1) units woven between S blocks so
            # the exp/mask (ACT/DVE) load spreads over the whole timeline
            with tc.tile_pool(name="p2", bufs=1) as p2, \
                 tc.tile_pool(name="p4", bufs=1) as p4, \
                 tc.tile_pool(name="ps2", bufs=2, space="PSUM") as ps2, \
                 tc.tile_pool(name="pss", bufs=2, space="PSUM") as pss, \
                 tc.tile_pool(name="pso", bufs=4, space="PSUM") as pso:
                wts = {}

                def wqk_load(hb):
                    wtall = p2.tile([128, 8, 128], dt.bfloat16, tag="wqk",
                                    bufs=5)
                    nc.sync.dma_start(out=wtall[:], in_=wqk[:, hb, :, :])
                    wts[hb] = wtall

                def wq8_load(hb):
                    t = p2.tile([128, 4, 2, 128], dt.float8e4, tag="wq8",
                                bufs=4)
                    nc.sync.dma_start(out=t[:], in_=wq8[:, hb, :, :, :])
                    wts[hb] = t

                def p2unit(hb, s):
                    wt = wts[hb]
                    g, h = (hb % 8) // 2, hb % 2
                    dest = QT[g] if hb < 8 else KT[g]
                    pps = ps2.tile([128, 512], dt.float32, tag="qkps")
                    if hb < 8:
                        # fp8 DoubleRow: W (x32) @ x (x8); undo 256x at the copy
                        for c in range(4):
                            nc.tensor.matmul(
                                pps[:], wt[:, c, :, :], qtb8t[:, s, c, :, :],
                                start=(c == 0), stop=(c == 3),
                                perf_mode=mybir.MatmulPerfMode.DoubleRow)
                        nc.scalar.activation(dest[:, h, s * 512:(s + 1) * 512],
                                             pps[:], AF.Identity,
                                             bias=bqk_t[:, hb:hb + 1],
                                             scale=1.0 / 256.0)
                    else:
                        for hin in range(8):
                            nc.tensor.matmul(pps[:], wt[:, hin, :], qtbst[s][hin],
                                             start=(hin == 0), stop=(hin == 7))
                        nc.scalar.activation(dest[:, h, s * 512:(s + 1) * 512],
                                             pps[:], AF.Identity,
                                             bias=bqk_t[:, hb:hb + 1], scale=1.0)
                    if s == 3:
                        wts.pop(hb)

                def vunit(lb, h):
                    vps = ps2.tile([128, 512], dt.float32, tag="qkps")
                    qs, qc = lb // 4, (lb % 4) * 128
                    for hin in range(8):
                        nc.tensor.matmul(vps[:],
                                         qtbst[qs][hin][:, qc:qc + 128],
                                         wv_t[hin][:, h * 512:(h + 1) * 512],
                                         start=(hin == 0), stop=(hin == 7))
                    for gg in range(2):
                        g2 = h * 2 + gg
                        nc.vector.tensor_tensor(
                            out=VT[lb][:, g2 * (D + 1):g2 * (D + 1) + D],
                            in0=vps[:, gg * D:(gg + 1) * D],
                            in1=bvb[:, g2 * D:(g2 + 1) * D], op=Alu.add)

                def emit(u):
                    if u[0] == "load":
                        (wq8_load if u[1] < 8 else wqk_load)(u[1])
                    elif u[0] == "unit":
                        p2unit(u[1], u[2])
                    else:
                        vunit(u[1], u[2])

                ghb = [[2 * g, 2 * g + 1, 8 + 2 * g, 8 + 2 * g + 1]
                       for g in range(G)]

                # ---- QK proj for group 0: fp8 Q units first (their inputs
                # arrive first), K units staggered behind the bf16 strip dma
                wq8_load(0)
                wq8_load(1)
                for s in range(2):
                    p2unit(0, s)
                    p2unit(1, s)
                wqk_load(8)
                wqk_load(9)
                for s in range(2, 4):
                    p2unit(0, s)
                    p2unit(1, s)
                for s in range(2):
                    p2unit(8, s)
                    p2unit(9, s)
                for s in range(2, 4):
                    p2unit(8, s)
                    p2unit(9, s)

                nc.gpsimd.partition_broadcast(wbig[:], wrow_t[:], channels=128)

                # ---- filler queue: V-proj + later groups' QK proj, with
                # deadlines (unit must be emitted before deadline pair starts)
                fillq = []  # (deadline_pair_index, unit)

                def pidx(g, s):
                    return g * 4 + (3 - s)

                for lb in range(15, -1, -1):
                    for h in range(2):
                        # VT[lb] consumed from pair (0, lb//4) onward
                        fillq.append((pidx(0, lb // 4), ("v", lb, h)))
                for gg in range(1, G):
                    dl = pidx(gg, 3)
                    h2 = ghb[gg]
                    fillq.append((dl, ("load", h2[0])))
                    fillq.append((dl, ("load", h2[1])))
                    for idx, hb in enumerate(h2):
                        if idx + 2 < 4:
                            fillq.append((dl, ("load", h2[idx + 2])))
                        for s in range(4):
                            fillq.append((dl, ("unit", hb, s)))

                ZCAP = 2048
                for g in range(G):
                    for s in (3, 2, 1, 0):
                        # force-emit fillers due before this pair
                        while fillq and fillq[0][0] <= pidx(g, s):
                            emit(fillq.pop(0)[1])

                        q0 = s * 512
                        amaxs, nmax, diag_safe = plan[(g, s)]
                        amax = max(amaxs)
                        outps = [pso.tile([128, 512], dt.float32, tag="outps",
                                          name="outps") for _ in range(4)]

                        def epilogue(j):
                            c = s * 4 + j
                            rden = p4.tile([128, 1], dt.float32, tag="rden",
                                           bufs=4, name="rden")
                            nc.vector.reciprocal(out=rden[:],
                                                 in_=outps[j][:, D:D + 1])
                            outn = p4.tile([128, D], dt.float32, tag="outn",
                                           bufs=4, name="outn")
                            if j % 2 == 0:
                                nc.scalar.mul(outn[:], outps[j][:, :D], rden[:])
                            else:
                                nc.vector.tensor_scalar(
                                    out=outn[:], in0=outps[j][:, :D],
                                    scalar1=rden[:], scalar2=None, op0=Alu.mult)
                            nc.sync.dma_start(
                                out=out[c * 128:(c + 1) * 128, g * D:(g + 1) * D],
                                in_=outn[:])

                        def consume(kb, parts):
                            for (pt, pcol, cb, jlo, jhi) in parts:
                                for j in range(jlo, jhi + 1):
                                    if 4 * s + j <= kb <= amaxs[j]:
                                        nc.tensor.matmul(
                                            outps[j][:, :D + 1],
                                            pt[:, pcol + j * 128 - cb:
                                               pcol + (j + 1) * 128 - cb],
                                            VT[kb][:, g * (D + 1):(g + 1) * (D + 1)],
                                            start=(kb == amaxs[j]),
                                            stop=(kb == 4 * s + j))
                            if kb < 4 * s + 4:
                                epilogue(kb - 4 * s)

                        def width(kb):
                            return 512 if kb >= 4 * s + 3 else (kb - 4 * s + 1) * 128

                        def loffset(kb):
                            # first live q-block for this key block
                            for j in range(4):
                                if amaxs[j] >= kb:
                                    return j * 128
                            raise AssertionError((g, s, kb, amaxs))

                        pending = []
                        z2 = None
                        zoff = 0
                        zrec = []

                        def flush_pack():
                            nonlocal z2
                            pt2 = p4.tile([128, ZCAP], dt.bfloat16, tag="pt",
                                          bufs=3, name="pt2")
                            nc.scalar.activation(pt2[:, :zoff], z2[:, :zoff],
                                                 AF.Exp, scale=SCALE2)
                            for kbx, zox, cbx, jlo, jhi, extra in zrec:
                                pending.append(
                                    (kbx, extra + [(pt2, zox, cbx, jlo, jhi)]))
                            z2 = None

                        def pack_room(need):
                            # flush if the pack can't fit `need` more columns
                            nonlocal z2, zoff, zrec
                            if z2 is not None and zoff + need > ZCAP:
                                flush_pack()
                            if z2 is None:
                                z2 = p4.tile([128, ZCAP], dt.bfloat16, tag="z",
                                             bufs=2, name="z2")
                                zoff, zrec = 0, []

                        for kb in range(amax, 4 * s - 1, -1):
                            w = width(kb)
                            off = loffset(kb)
                            jmin = off // 128
                            sps = pss.tile([128, 512], dt.float32, tag="sps")
                            nc.tensor.matmul(sps[:, off:w],
                                             KT[g][:, :, kb * 128:(kb + 1) * 128],
                                             QT[g][:, :, q0 + off:q0 + w],
                                             start=True, stop=True,
                                             perf_mode=mybir.MatmulPerfMode.DoubleRow)
                            if len(pending) >= 4:
                                consume(*pending.pop(0))
                            # paced filler: at most one group ahead of deadline
                            if fillq and fillq[0][0] <= pidx(g, s) + 5:
                                emit(fillq.pop(0)[1])
                            if 4 * s + 4 <= kb <= nmax:
                                # fully inside the window: no mask needed
                                if z2 is not None:
                                    flush_pack()
                                pt1 = p4.tile([128, 512], dt.bfloat16, tag="pt1",
                                              bufs=4, name="pt1")
                                nc.scalar.activation(pt1[:, off:w], sps[:, off:w],
                                                     AF.Exp, scale=SCALE2)
                                pending.append((kb, [(pt1, 0, 0, jmin, 3)]))
                                continue
                            if kb > nmax:
                                # boundary: windowed mask on all live columns
                                lw = w - off
                                pack_room(lw)
                                nc.vector.scalar_tensor_tensor(
                                    z2[:, zoff:zoff + lw], wbig[:, q0 + off:q0 + w],
                                    kvec_t[:, g * 16 + kb:g * 16 + kb + 1],
                                    sps[:, off:w], op0=Alu.subtract, op1=Alu.min)
                                zrec.append((kb, zoff, off, jmin, 3, []))
                                zoff += lw
                                if kb == 4 * s:
                                    flush_pack()
                                continue
                            # near-diagonal (kb <= 4s+3): upper window can't bind
                            # (host-checked diag_safe); only the triangular 128
                            # needs masking.
                            jdiag = kb - 4 * s
                            if diag_safe:
                                extra = []
                                if w - off > 128:
                                    pt1 = p4.tile([128, 512], dt.bfloat16,
                                                  tag="pt1", bufs=4, name="pt1")
                                    nc.scalar.activation(pt1[:, off:w - 128],
                                                         sps[:, off:w - 128],
                                                         AF.Exp, scale=SCALE2)
                                    extra.append((pt1, 0, 0, jmin, jdiag - 1))
                                pack_room(128)
                                nc.vector.tensor_tensor(
                                    out=z2[:, zoff:zoff + 128],
                                    in0=sps[:, w - 128:w], in1=dt_t[:], op=Alu.min)
                                zrec.append((kb, zoff, w - 128, jdiag, jdiag, extra))
                                zoff += 128
                            else:
                                lw = w - off
                                pack_room(lw)
                                nc.vector.scalar_tensor_tensor(
                                    z2[:, zoff:zoff + lw], wbig[:, q0 + off:q0 + w],
                                    kvec_t[:, g * 16 + kb:g * 16 + kb + 1],
                                    sps[:, off:w], op0=Alu.subtract, op1=Alu.min)
                                nc.vector.tensor_tensor(
                                    out=z2[:, zoff + lw - 128:zoff + lw],
                                    in0=z2[:, zoff + lw - 128:zoff + lw],
                                    in1=dt_t[:], op=Alu.min)
                                zrec.append((kb, zoff, off, jmin, jdiag, []))
                                zoff += lw
                            if kb == 4 * s:
                                flush_pack()
                        if z2 is not None:
                            flush_pack()
                        for it in pending:
                            consume(*it)

    nc.finalize()
    return nc


def _make_plan(query, woffl_np, lin2_b):
    """Host-exact window offsets -> per-(g,s) block plan (batch-uniform)."""
    z = np.maximum(query.astype(np.float64), 0.0).reshape(-1, H) @ woffl_np
    dx = (1.0 / (1.0 + np.exp(-(z + float(lin2_b[0]))))).reshape(B, L) * L
    plan = {}
    q_idx = np.arange(L, dtype=np.float64)
    for g, ws in enumerate(WS):
        lim = q_idx[None, :] + dx + ws          # [B, L] max allowed k (float)
        amax_qb = []
        for qb in range(16):
            sl = lim[:, qb * 128:(qb + 1) * 128]
            a = qb
            for kb in range(15, qb - 1, -1):
                if not (kb * 128 > sl + MARGIN).all():
                    a = kb
                    break
            amax_qb.append(a)
        for s in range(4):
            amaxs = tuple(amax_qb[4 * s:4 * s + 4])
            sl = lim[:, s * 512:(s + 1) * 512]
            nmax = 4 * s + 3
            for kb in range(min(max(amaxs), 15), 4 * s + 3, -1):
                if (kb * 128 + 127 <= sl - MARGIN).all():
                    nmax = kb
                    break
            # interior (no-mask) blocks must be live for every q-block
            assert nmax == 4 * s + 3 or nmax <= min(amaxs), (g, s, amaxs, nmax)
            # near-diagonal blocks (kb<=4s+3, k-q<=511) can skip the upper
            # window test iff the window covers >=511+MARGIN keys for every row
            diag_safe = bool(
                (dx[:, s * 512:(s + 1) * 512] + ws > 511 + MARGIN).all())
            plan[(g, s)] = (amaxs, nmax, diag_safe)
    return plan, dx


def _prep_shared(qkv_w, qkv_b, off_w, lin2_w, lin2_b):
    f32 = np.float32
    bf = ml_dtypes.bfloat16
    qkv_wT = np.ascontiguousarray(qkv_w.T, dtype=f32)          # [H, 3H]
    woffl = (off_w.T.astype(np.float64) @ lin2_w.T.astype(np.float64))  # [H, 1]
    # [H, 2H] -> [p, hb, hin, c]; [H, H] -> [p, hin, c]  (partition-major)
    wqk_np = (qkv_wT[:, :2 * H].reshape(8, 128, 16, 128)
              .transpose(1, 2, 0, 3))
    wv_np = qkv_wT[:, 2 * H:].reshape(8, 128, H).transpose(1, 0, 2)
    # fp8 Q-projection weights, x32 so values clear the e4m3 denormal range
    wq8_np = ((qkv_wT[:, :H] * 32.0).reshape(4, 2, 128, 8, 128)
              .transpose(2, 3, 0, 1, 4))
    p = np.arange(128, dtype=np.float64)[:, None]
    cols = []
    for g in range(G):
        for kb in range(16):
            cols.append(BIG * (kb * 128 + p - WS[g]))
    kvec = np.concatenate(cols, axis=1).astype(f32)
    bqk = np.ascontiguousarray(qkv_b[:2 * H].reshape(16, 128).T, dtype=f32)
    pi = np.arange(128)[:, None]
    fi = np.arange(128)[None, :]
    dtile = np.where(pi >= fi, 1e6, -1e6).astype(f32)
    bv = np.broadcast_to(qkv_b[2 * H:][None], (128, H))
    woffl_col = woffl.reshape(8, 128).T
    iotab = BIG * np.arange(L, dtype=np.float64)
    shared = {
        "wqk": np.ascontiguousarray(wqk_np).astype(bf),
        "wq8": np.ascontiguousarray(wq8_np).astype(ml_dtypes.float8_e4m3),
        "wv": np.ascontiguousarray(wv_np).astype(bf),
        "cf32": np.concatenate([bqk, kvec], axis=1).astype(f32),
        "cbf": np.concatenate([dtile, bv], axis=1).astype(bf),
    }
    return shared, woffl


def kernel(query, key_in, value, qkv_w, qkv_b, off_w, lin2_w, lin2_b,
           _trace=False, _tmpdir=None):
    query = np.asarray(query, dtype=np.float32)
    shared, woffl_np = _prep_shared(np.asarray(qkv_w, np.float32),
                                    np.asarray(qkv_b, np.float32),
                                    np.asarray(off_w, np.float32),
                                    np.asarray(lin2_w, np.float32),
                                    np.asarray(lin2_b, np.float32))
    plan, dx = _make_plan(query, woffl_np, np.asarray(lin2_b, np.float64).ravel())
    in_maps = []
    for b in range(NCORES):
        m = dict(shared)
        # [p, strip, hin, col] so each 512-col strip is one contiguous dma run
        qT = (query[b].T.reshape(8, 128, 4, 512).transpose(1, 2, 0, 3))
        m["qtbf"] = np.ascontiguousarray(qT).astype(ml_dtypes.bfloat16)
        qT8 = ((query[b].T * 8.0).reshape(4, 2, 128, 4, 512)
               .transpose(2, 3, 0, 1, 4))
        m["qtb8"] = np.ascontiguousarray(qT8).astype(ml_dtypes.float8_e4m3)
        m["wrow"] = (BIG * (np.arange(L, dtype=np.float64) + dx[b])
                     ).astype(np.float32)[None]
        in_maps.append(m)

    key = tuple(sorted(plan.items()))
    if key not in _CACHE:
        _CACHE[key] = build_nc(plan)
    kw = {}
    if _trace:
        kw = dict(trace=True, tmpdir=_tmpdir)
    res = run_bass_kernel_spmd(_CACHE[key], in_maps,
                               core_ids=list(range(NCORES)), **kw)
    out = np.stack([np.asarray(res.results[b]["out"]) for b in range(NCORES)],
                   axis=0)
    if _trace:
        return out, res
    return out


if __name__ == "__main__":
    rng = np.random.default_rng(0)
    ins = {
        "query": rng.standard_normal((B, L, H)).astype(np.float32),
        "key_in": rng.standard_normal((B, L, H)).astype(np.float32),
        "value": rng.standard_normal((B, L, H)).astype(np.float32),
        "qkv_w": (rng.standard_normal((3 * H, H)) * 0.02).astype(np.float32),
        "qkv_b": np.zeros(3 * H, np.float32),
        "off_w": (rng.standard_normal((D1, H)) * 0.02).astype(np.float32),
        "lin2_w": (rng.standard_normal((1, D1)) * 0.02).astype(np.float32),
        "lin2_b": np.zeros(1, np.float32),
    }
    o = kernel(**ins)
    print("out", o.shape, o.dtype, np.abs(o).mean())

